# revision 3
# baseline (speedup 1.0000x reference)
"""CWSA (channel-wise self-attention) layer for Trainium2, 8 NeuronCores.

Math (per batch b of 4):
    x_q = W_qk @ x[b]                  # [64, 4096]   (k == q, tied weights)
    x_v = W_v  @ x[b] + b_v            # [64, 4096]
    E   = x_q^T x_q / 8                # [4096, 4096] Gram matrix
    A   = softmax(E, axis=-1)          # rows sum to 1
    out = x_v @ A                      # [64, 4096]
Sharding: 8 cores = 4 batches x 2 halves of the n (row/contraction) axis;
host sums the two partial outputs per batch.

The kernel is a single exp stream on the scalar engine (the bottleneck:
64 x [128,1024] chunks ~ 1.1us each); everything else hides under it.

Differences from the first working version (107us):
  * rowsums via one DVE tensor_scalar(mult 1.0, accum_out) per tile at 4x
    perf mode (~1.1us) instead of scalar accum reads (64 x ~290ns on the
    exp-critical scalar queue) + 32 slow 1x tensor_reduces.
  * ramp: input DMAs are column-chunked and queue-FIFO-prioritized (low
    half first on both rings, one DMA per chunk covering both row halves),
    the first two exp chunks are 512 wide, and tiles 0/1 are interleaved
    (low-column chunks of both first) so the hi-half DMA+projection
    latency hides under low-half exps.
  * tail: the last tile accumulates its rowsum per chunk, AV runs at 512
    wide in bank order, and each PSUM bank is copied + DMA'd out as soon
    as it closes (staggered over sync/gpsimd/scalar queues).
"""

import sys

sys.path.insert(0, "/opt/trn_rl_repo")

import numpy as np
import ml_dtypes

import concourse.bass as bass
import concourse.mybir as mybir
import concourse.tile as tile
from concourse import bacc
from concourse.bass import ts, ds

B = 4
C = 256
C4 = 64
N = 4096
NH = N // 2          # n rows per core
NT = 128             # n-tile rows
NTILES = NH // NT    # 16
FACTOR = float(np.sqrt(C4))  # 8.0

BF16 = mybir.dt.bfloat16
F32 = mybir.dt.float32
EXP = mybir.ActivationFunctionType.Exp
ADD = mybir.AluOpType.add
MULT = mybir.AluOpType.mult


def build_nc() -> bass.Bass:
    nc = bacc.Bacc("TRN2", target_bir_lowering=False, debug=False, num_devices=8)

    x_m = nc.declare_dram_parameter("x_m", [C, N], BF16, isOutput=False)
    wq_t = nc.declare_dram_parameter("wq_t", [C, C4], BF16, isOutput=False)
    wv_t = nc.declare_dram_parameter("wv_t", [C, C4], BF16, isOutput=False)
    bv = nc.declare_dram_parameter("bv", [C4], BF16, isOutput=False)
    out_p = nc.declare_dram_parameter("out_p", [C4, N], F32, isOutput=True)

    from contextlib import ExitStack

    with tile.TileContext(nc) as tc, ExitStack() as ctx:
        sing = ctx.enter_context(tc.tile_pool(name="sing", bufs=1))
        small = ctx.enter_context(tc.tile_pool(name="small", bufs=4))
        work = ctx.enter_context(tc.tile_pool(name="work", bufs=6))
        e_ps = ctx.enter_context(tc.tile_pool(name="e_ps", bufs=2, space="PSUM"))
        xr_ps = ctx.enter_context(tc.tile_pool(name="xr_ps", bufs=1, space="PSUM"))

        # ---- input loads -------------------------------------------------
        # The host rotates x[b] per core so the local n-half is always
        # columns 0:2048. Each x DMA covers BOTH 128-row halves of one
        # column chunk (3D access pattern), so a projection becomes ready
        # the moment its single chunk lands. Chunks are issued low-half
        # first on both rings; ring FIFO order gives the low half strict
        # SDMA priority over the hi half.
        xm_sb = sing.tile([128, 2, N], BF16)
        wq_sb = sing.tile([128, 2, C4], BF16)
        wv_sb = sing.tile([128, 2, C4], BF16)
        bv_bc = sing.tile([128, C4], BF16)

        def x_src(col0, w):
            ap = x_m[:]
            return bass.AP(
                tensor=ap.tensor,
                offset=col0,
                ap=[[N, 128], [N * 128, 2], [1, w]],
            )

        def w_src(w_t):
            ap = w_t[:]
            return bass.AP(
                tensor=ap.tensor,
                offset=0,
                ap=[[C4, 128], [C4 * 128, 2], [1, C4]],
            )

        nc.sync.dma_start(out=wq_sb, in_=w_src(wq_t))
        nc.sync.dma_start(out=xm_sb[:, :, 0:512], in_=x_src(0, 512))
        nc.sync.dma_start(out=xm_sb[:, :, 512:1024], in_=x_src(512, 512))
        nc.sync.dma_start(out=xm_sb[:, :, 2048:3072], in_=x_src(2048, 1024))
        bv_ap = bv[:]
        bv_bcast = bass.AP(
            tensor=bv_ap.tensor, offset=bv_ap.offset, ap=[[0, 128]] + list(bv_ap.ap)
        )
        nc.sync.dma_start(out=bv_bc, in_=bv_bcast)
        # wv first on gpsimd (tiny, and the v path needs it mid-stream)
        nc.gpsimd.dma_start(out=wv_sb, in_=w_src(wv_t))
        nc.gpsimd.dma_start(out=xm_sb[:, :, 1024:1536], in_=x_src(1024, 512))
        nc.gpsimd.dma_start(out=xm_sb[:, :, 1536:2048], in_=x_src(1536, 512))
        nc.gpsimd.dma_start(out=xm_sb[:, :, 3072:4096], in_=x_src(3072, 1024))

        # ---- PE warm-up --------------------------------------------------
        # back-to-back dummy matmuls under the input DMAs so the PE HAM
        # p-state climbs before the projections start. gpsimd memset: the
        # pool engine boots earliest, keeping vector free for casts.
        warm_in = sing.tile([128, 512], BF16)
        nc.gpsimd.memset(warm_in, 0.0)
        warm_ps = e_ps.tile([128, 512], F32, tag="e", name="warm_ps")
        for i in range(6):
            nc.tensor.matmul(warm_ps, warm_in[:, 0:128], warm_in,
                             start=True, stop=True)

        # ---- projections -------------------------------------------------
        # q is stored twice along partitions (0:64 and 64:128) so energy
        # fills can row-slot-pack two K=64 matmuls into the PE array.
        def colpack_proj(dst_ps, rhs0, rhs1):
            nc.tensor.matmul(dst_ps[0:64, :], wq_sb[:, 0, :], rhs0,
                             start=True, stop=False, tile_position=(0, 0))
            nc.tensor.matmul(dst_ps[64:128, :], wq_sb[:, 0, :], rhs0,
                             start=True, stop=False, tile_position=(0, 64),
                             skip_group_check=True)
            nc.tensor.matmul(dst_ps[0:64, :], wq_sb[:, 1, :], rhs1,
                             start=False, stop=True, tile_position=(0, 0))
            nc.tensor.matmul(dst_ps[64:128, :], wq_sb[:, 1, :], rhs1,
                             start=False, stop=True, tile_position=(0, 64),
                             skip_group_check=True)

        xqt = [sing.tile([128, 1024], BF16, name=f"xq{i}") for i in range(4)]

        def xk(row, t):
            i, off = (t * NT) // 1024, (t * NT) % 1024
            return xqt[i][row:row + 64, off:off + NT]

        def xq(row, col, w):
            i, cc = col // 1024, col % 1024
            return xqt[i][row:row + 64, cc:cc + w]

        def q_proj(j):
            qp = xr_ps.tile([128, 512], F32, tag=f"xr{j % 4}", name=f"qp{j}")
            colpack_proj(qp, xm_sb[:, 0, ts(j, 512)], xm_sb[:, 1, ts(j, 512)])
            dst = xqt[j // 2][:, (j % 2) * 512:(j % 2) * 512 + 512]
            # q1's cast rides the scalar queue (before any exp) so q0/q1
            # casts run on two engines in parallel for the earliest fills.
            if j == 1:
                nc.scalar.copy(out=dst, in_=qp)
            else:
                nc.vector.tensor_copy(out=dst, in_=qp)

        # ---- energy fill / exp plumbing ----------------------------------
        # stream order: tiles 0/1 interleaved at the front so the hi-half
        # (cols 2048:4096) DMA + projection latency hides under low-half
        # exp chunks.
        chunk_list = [(0, 0), (0, 1), (1, 0), (1, 1),
                      (0, 2), (0, 3), (1, 2), (1, 3)]
        for t in range(2, NTILES):
            chunk_list += [(t, 0), (t, 1), (t, 2), (t, 3)]

        def emit_fill(t, c):
            e_t = e_ps.tile([128, 1024], F32, tag="e", name=f"e{t}_{c}")
            m0 = 1024 * c
            nc.tensor.matmul(e_t[:, 0:512], xk(0, t), xq(0, m0, 512),
                             start=True, stop=True, tile_position=(0, 0))
            nc.tensor.matmul(e_t[:, 512:1024], xk(64, t), xq(64, m0 + 512, 512),
                             start=True, stop=True, tile_position=(64, 0),
                             skip_group_check=True)
            return e_t

        # prologue: projections and the first two fills, interleaved so
        # each fill is emitted as soon as its q columns exist.
        q_proj(0)
        q_proj(1)
        etiles = {(0, 0): emit_fill(0, 0)}
        q_proj(2)
        q_proj(3)
        etiles[(0, 1)] = emit_fill(0, 1)
        for j in range(4, 8):
            q_proj(j)

        # per-tile v projections (deprioritized PE gap filler)
        xvt_sb = [
            sing.tile([128, C4], BF16, name=f"xvt{t}") for t in range(NTILES)
        ]
        for t in range(NTILES):
            vp = xr_ps.tile([128, C4], F32, tag=f"xr{t % 4}", name=f"vp{t}")
            half = t // 8
            off = (t % 8) * NT
            mm1 = nc.tensor.matmul(vp, xm_sb[:, 0, ds(half * 1024 + off, NT)],
                                   wv_sb[:, 0, :], start=True, stop=False)
            mm2 = nc.tensor.matmul(vp, xm_sb[:, 1, ds(half * 1024 + off, NT)],
                                   wv_sb[:, 1, :], start=False, stop=True)
            mm1.ins.bass_priority = 500_000 + 2 * t
            mm2.ins.bass_priority = 500_000 + 2 * t + 1
            nc.vector.tensor_add(out=xvt_sb[t], in0=vp, in1=bv_bc)

        # ---- output accumulators (partition-packed: even m-chunk in
        # partitions 0-63, odd in 64-127) -----------------------------------
        xr = [
            xr_ps.tile([128, 512], F32, tag=f"xr{k}", name=f"xr{k}")
            for k in range(4)
        ]

        p_tiles = {}
        rs4 = small.tile([128, 4], F32, tag="rs4")

        def do_exp(t, c):
            p = p_tiles[t]
            e_t = etiles.pop((t, c))
            if (t, c) in ((0, 0), (0, 1)):
                # split so the very first exp depends on a single 512-col
                # fill (subtile deps): the stream starts ~2 fills earlier.
                nc.scalar.activation(out=p[:, ds(1024 * c, 512)],
                                     in_=e_t[:, 0:512], func=EXP)
                nc.scalar.activation(out=p[:, ds(1024 * c + 512, 512)],
                                     in_=e_t[:, 512:1024], func=EXP)
            else:
                nc.scalar.activation(out=p[:, ds(1024 * c, 1024)], in_=e_t,
                                     func=EXP)
            if t == NTILES - 1:
                # tail: accumulate the rowsum per chunk so the final
                # normalization starts ~330ns (not ~1.2us) after the last
                # exp. in-place mult-by-1 with accum_out runs at 4x.
                nc.vector.tensor_scalar(
                    out=p[:, ds(1024 * c, 1024)], in0=p[:, ds(1024 * c, 1024)],
                    scalar1=1.0, scalar2=None, op0=MULT, op1=ADD,
                    accum_out=rs4[:, c:c + 1])

        def finish_tile(t):
            p = p_tiles[t]
            rs = small.tile([128, 1], F32, tag="rs")
            if t == NTILES - 1:
                nc.vector.tensor_scalar(out=rs4, in0=rs4, scalar1=1.0,
                                        scalar2=None, op0=MULT, op1=ADD,
                                        accum_out=rs)
            else:
                # one 4x-mode pass over the whole 4096-wide row: rowsum in
                # a single DVE op, nothing on the scalar queue.
                nc.vector.tensor_scalar(out=p, in0=p, scalar1=1.0,
                                        scalar2=None, op0=MULT, op1=ADD,
                                        accum_out=rs)
            rr = small.tile([128, 1], F32, tag="rr")
            nc.vector.reciprocal(out=rr, in_=rs)
            xvs = small.tile([128, C4], BF16, tag="xvs")
            nc.vector.tensor_scalar_mul(out=xvs, in0=xvt_sb[t], scalar1=rr)

            first = t == 0
            last = t == NTILES - 1
            # t==0 uses 512-wide AV (start=True must cover the full 2KB
            # PSUM zero-region); mid tiles 256 so an in-flight AV matmul
            # delays a just-released energy fill by at most ~215ns; the
            # last tile uses 512 again (nothing left to delay) emitted in
            # bank order so banks close staggered for the epilogue.
            av_w = 512 if (first or last) else 256
            for j in range(8):
                k, po = j // 2, (j % 2) * 64
                for s in range(512 // av_w):
                    mm = nc.tensor.matmul(
                        xr[k][po:po + 64, ds(s * av_w, av_w)], xvs,
                        p[:, ds(j * 512 + s * av_w, av_w)],
                        start=first, stop=last, tile_position=(0, po),
                        skip_group_check=True,
                    )
                    if not last:
                        mm.ins.bass_priority = 1_000_000 + t * 100 + j * 4 + s

        # ---- the stream --------------------------------------------------
        for i, (t, c) in enumerate(chunk_list):
            if c == 0 or (t, c) == (0, 2):
                if t not in p_tiles:
                    p_tiles[t] = work.tile([128, N], BF16, tag="p",
                                           name=f"p{t}")
            do_exp(t, c)
            if i + 2 < len(chunk_list):
                nt_, nc_ = chunk_list[i + 2]
                etiles[(nt_, nc_)] = emit_fill(nt_, nc_)
            if c == 3:
                finish_tile(t)

        # ---- epilogue: per-bank staggered PSUM->SBUF copy + DMA ----------
        out_sb = sing.tile([128, 4, 512], F32)
        dma_engines = [nc.sync, nc.gpsimd, nc.scalar, nc.sync]
        for k in range(4):
            if k % 2 == 0:
                nc.scalar.copy(out=out_sb[:, k, :], in_=xr[k])
            else:
                nc.vector.tensor_copy(out=out_sb[:, k, :], in_=xr[k])
            eng = dma_engines[k]
            eng.dma_start(out=out_p[:, ts(2 * k, 512)], in_=out_sb[0:64, k, :])
            eng.dma_start(out=out_p[:, ts(2 * k + 1, 512)],
                          in_=out_sb[64:128, k, :])

    nc.compile()
    return nc


_NC_CACHE = None


def _get_nc():
    global _NC_CACHE
    if _NC_CACHE is None:
        _NC_CACHE = build_nc()
    return _NC_CACHE


def make_in_maps(x, W_qk, W_v, b_v):
    bf = ml_dtypes.bfloat16
    x = np.asarray(x, dtype=np.float32)
    W_qk = np.asarray(W_qk, dtype=np.float32)
    W_v = np.asarray(W_v, dtype=np.float32)
    b_v = np.asarray(b_v, dtype=np.float32)
    xbf = np.ascontiguousarray(x).astype(bf)
    wqt = np.ascontiguousarray((W_qk / np.sqrt(FACTOR)).T).astype(bf)
    wvt = np.ascontiguousarray(W_v.T).astype(bf)
    bvb = np.ascontiguousarray(b_v).astype(bf)
    in_maps = []
    for core in range(8):
        b, h = core // 2, core % 2
        xm = xbf[b] if h == 0 else np.ascontiguousarray(
            np.roll(xbf[b], -NH, axis=1))
        in_maps.append({
            "x_m": xm,
            "wq_t": wqt,
            "wv_t": wvt,
            "bv": bvb,
        })
    return in_maps


def kernel(x, W_qk, W_v, b_v, _trace=False):
    from concourse.bass_utils import run_bass_kernel_spmd

    nc = _get_nc()
    in_maps = make_in_maps(x, W_qk, W_v, b_v)
    res = run_bass_kernel_spmd(nc, in_maps, list(range(8)), trace=_trace)
    if _trace:
        print(f"HW exec time: {res.exec_time_ns} ns")
        print(f"mean exec time: {res.mean_exec_time_ns} ns")
    outs = [res.results[i]["out_p"] for i in range(8)]
    out = np.stack([
        outs[2 * b] + np.roll(outs[2 * b + 1], NH, axis=1) for b in range(B)
    ])
    return out.astype(np.float32)


# revision 8
# speedup vs baseline: 1.3850x; 1.3850x over previous
"""CWSA (channel-wise self-attention) layer for Trainium2, 8 NeuronCores.

Math (per batch b of 4):
    x_q = W_qk @ x[b]                  # [64, 4096]   (k == q, tied weights)
    x_v = W_v  @ x[b] + b_v            # [64, 4096]
    E   = x_q^T x_q / 8                # [4096, 4096] Gram matrix
    A   = softmax(E, axis=-1)          # rows sum to 1
    out = x_v @ A                      # [64, 4096]
Sharding: 8 cores = 4 batches x 2 halves of the n (row/contraction) axis;
host sums the two partial outputs per batch.

The kernel is a single exp stream on the scalar engine (the bottleneck:
64 x [128,1024] chunks ~ 1.1us each); everything else hides under it.

Differences from the first working version (107us):
  * rowsums via one DVE tensor_scalar(mult 1.0, accum_out) per tile at 4x
    perf mode (~1.1us) instead of scalar accum reads (64 x ~290ns on the
    exp-critical scalar queue) + 32 slow 1x tensor_reduces.
  * ramp: input DMAs are column-chunked and queue-FIFO-prioritized (low
    half first on both rings, one DMA per chunk covering both row halves),
    the first two exp chunks are 512 wide, and tiles 0/1 are interleaved
    (low-column chunks of both first) so the hi-half DMA+projection
    latency hides under low-half exps.
  * tail: the last tile accumulates its rowsum per chunk, AV runs at 512
    wide in bank order, and each PSUM bank is copied + DMA'd out as soon
    as it closes (staggered over sync/gpsimd/scalar queues).
"""

import sys

sys.path.insert(0, "/opt/trn_rl_repo")

import numpy as np
import ml_dtypes

import concourse.bass as bass
import concourse.mybir as mybir
import concourse.tile as tile
from concourse import bacc
from concourse.bass import ts, ds

B = 4
C = 256
C4 = 64
N = 4096
NH = N // 2          # n rows per core
NT = 128             # n-tile rows
NTILES = NH // NT    # 16
FACTOR = float(np.sqrt(C4))  # 8.0

BF16 = mybir.dt.bfloat16
F32 = mybir.dt.float32
EXP = mybir.ActivationFunctionType.Exp
ADD = mybir.AluOpType.add
MULT = mybir.AluOpType.mult


def build_nc() -> bass.Bass:
    nc = bacc.Bacc("TRN2", target_bir_lowering=False, debug=False, num_devices=8)

    x_m = nc.declare_dram_parameter("x_m", [C, N], BF16, isOutput=False)
    wq_t = nc.declare_dram_parameter("wq_t", [C, C4], BF16, isOutput=False)
    wv_t = nc.declare_dram_parameter("wv_t", [C, C4], BF16, isOutput=False)
    bv = nc.declare_dram_parameter("bv", [C4], BF16, isOutput=False)
    out_p = nc.declare_dram_parameter("out_p", [C4, N], F32, isOutput=True)

    from contextlib import ExitStack

    with tile.TileContext(nc) as tc, ExitStack() as ctx:
        sing = ctx.enter_context(tc.tile_pool(name="sing", bufs=1))
        small = ctx.enter_context(tc.tile_pool(name="small", bufs=4))
        work = ctx.enter_context(tc.tile_pool(name="work", bufs=6))
        e_ps = ctx.enter_context(tc.tile_pool(name="e_ps", bufs=2, space="PSUM"))
        xr_ps = ctx.enter_context(tc.tile_pool(name="xr_ps", bufs=1, space="PSUM"))

        # ---- input loads -------------------------------------------------
        # The host rotates x[b] per core so the local n-half is always
        # columns 0:2048. Each x DMA covers BOTH 128-row halves of one
        # column chunk (3D access pattern), so a projection becomes ready
        # the moment its single chunk lands. Chunks are issued low-half
        # first on both rings; ring FIFO order gives the low half strict
        # SDMA priority over the hi half.
        xm_sb = sing.tile([128, 2, N], BF16)
        wq_sb = sing.tile([128, 2, C4], BF16)
        wv_sb = sing.tile([128, 2, C4], BF16)
        bv_bc = sing.tile([128, C4], BF16)

        def x_src(col0, w):
            ap = x_m[:]
            return bass.AP(
                tensor=ap.tensor,
                offset=col0,
                ap=[[N, 128], [N * 128, 2], [1, w]],
            )

        def w_src(w_t):
            ap = w_t[:]
            return bass.AP(
                tensor=ap.tensor,
                offset=0,
                ap=[[C4, 128], [C4 * 128, 2], [1, C4]],
            )

        nc.sync.dma_start(out=wq_sb, in_=w_src(wq_t))
        nc.sync.dma_start(out=xm_sb[:, :, 0:512], in_=x_src(0, 512))
        nc.sync.dma_start(out=xm_sb[:, :, 512:1024], in_=x_src(512, 512))
        nc.sync.dma_start(out=xm_sb[:, :, 2048:3072], in_=x_src(2048, 1024))
        bv_ap = bv[:]
        bv_bcast = bass.AP(
            tensor=bv_ap.tensor, offset=bv_ap.offset, ap=[[0, 128]] + list(bv_ap.ap)
        )
        nc.sync.dma_start(out=bv_bc, in_=bv_bcast)
        # wv first on gpsimd (tiny, and the v path needs it mid-stream)
        nc.gpsimd.dma_start(out=wv_sb, in_=w_src(wv_t))
        nc.gpsimd.dma_start(out=xm_sb[:, :, 1024:1536], in_=x_src(1024, 512))
        nc.gpsimd.dma_start(out=xm_sb[:, :, 1536:2048], in_=x_src(1536, 512))
        nc.gpsimd.dma_start(out=xm_sb[:, :, 3072:4096], in_=x_src(3072, 1024))

        # ---- PE warm-up --------------------------------------------------
        # back-to-back dummy matmuls under the input DMAs so the PE HAM
        # p-state climbs before the projections start. gpsimd memset: the
        # pool engine boots earliest, keeping vector free for casts.
        warm_in = sing.tile([128, 512], BF16)
        nc.gpsimd.memset(warm_in, 0.0)
        warm_ps = e_ps.tile([128, 512], F32, tag="e", name="warm_ps")
        for i in range(6):
            nc.tensor.matmul(warm_ps, warm_in[:, 0:128], warm_in,
                             start=True, stop=True)

        # ---- projections -------------------------------------------------
        # q is stored twice along partitions (0:64 and 64:128) so energy
        # fills can row-slot-pack two K=64 matmuls into the PE array.
        def colpack_proj(dst_ps, rhs0, rhs1):
            nc.tensor.matmul(dst_ps[0:64, :], wq_sb[:, 0, :], rhs0,
                             start=True, stop=False, tile_position=(0, 0))
            nc.tensor.matmul(dst_ps[64:128, :], wq_sb[:, 0, :], rhs0,
                             start=True, stop=False, tile_position=(0, 64),
                             skip_group_check=True)
            nc.tensor.matmul(dst_ps[0:64, :], wq_sb[:, 1, :], rhs1,
                             start=False, stop=True, tile_position=(0, 0))
            nc.tensor.matmul(dst_ps[64:128, :], wq_sb[:, 1, :], rhs1,
                             start=False, stop=True, tile_position=(0, 64),
                             skip_group_check=True)

        xqt = [sing.tile([128, 1024], BF16, name=f"xq{i}") for i in range(4)]

        def xk(row, t):
            i, off = (t * NT) // 1024, (t * NT) % 1024
            return xqt[i][row:row + 64, off:off + NT]

        def xq(row, col, w):
            i, cc = col // 1024, col % 1024
            return xqt[i][row:row + 64, cc:cc + w]

        def q_proj(j):
            qp = xr_ps.tile([128, 512], F32, tag=f"xr{j % 4}", name=f"qp{j}")
            colpack_proj(qp, xm_sb[:, 0, ts(j, 512)], xm_sb[:, 1, ts(j, 512)])
            dst = xqt[j // 2][:, (j % 2) * 512:(j % 2) * 512 + 512]
            # q1's cast rides the scalar queue (before any exp) so q0/q1
            # casts run on two engines in parallel for the earliest fills.
            if j == 1:
                nc.scalar.copy(out=dst, in_=qp)
            else:
                nc.vector.tensor_copy(out=dst, in_=qp)

        # ---- energy fill / exp plumbing ----------------------------------
        # stream order: tiles 0/1 interleaved at the front so the hi-half
        # (cols 2048:4096) DMA + projection latency hides under low-half
        # exp chunks.
        chunk_list = [(0, 0), (0, 1), (1, 0), (1, 1),
                      (0, 2), (0, 3), (1, 2), (1, 3)]
        for t in range(2, NTILES):
            chunk_list += [(t, 0), (t, 1), (t, 2), (t, 3)]

        def emit_fill(t, c):
            e_t = e_ps.tile([128, 1024], F32, tag="e", name=f"e{t}_{c}")
            m0 = 1024 * c
            nc.tensor.matmul(e_t[:, 0:512], xk(0, t), xq(0, m0, 512),
                             start=True, stop=True, tile_position=(0, 0))
            nc.tensor.matmul(e_t[:, 512:1024], xk(64, t), xq(64, m0 + 512, 512),
                             start=True, stop=True, tile_position=(64, 0),
                             skip_group_check=True)
            return e_t

        # prologue: projections and the first two fills, interleaved so
        # each fill is emitted as soon as its q columns exist.
        q_proj(0)
        q_proj(1)
        etiles = {(0, 0): emit_fill(0, 0)}
        q_proj(2)
        q_proj(3)
        etiles[(0, 1)] = emit_fill(0, 1)
        for j in range(4, 8):
            q_proj(j)

        # per-tile v projections (deprioritized PE gap filler)
        xvt_sb = [
            sing.tile([128, C4], BF16, name=f"xvt{t}") for t in range(NTILES)
        ]
        for t in range(NTILES):
            vp = xr_ps.tile([128, C4], F32, tag=f"xr{t % 4}", name=f"vp{t}")
            half = t // 8
            off = (t % 8) * NT
            mm1 = nc.tensor.matmul(vp, xm_sb[:, 0, ds(half * 1024 + off, NT)],
                                   wv_sb[:, 0, :], start=True, stop=False)
            mm2 = nc.tensor.matmul(vp, xm_sb[:, 1, ds(half * 1024 + off, NT)],
                                   wv_sb[:, 1, :], start=False, stop=True)
            mm1.ins.bass_priority = 500_000 + 2 * t
            mm2.ins.bass_priority = 500_000 + 2 * t + 1
            nc.vector.tensor_add(out=xvt_sb[t], in0=vp, in1=bv_bc)

        # ---- output accumulators (partition-packed: even m-chunk in
        # partitions 0-63, odd in 64-127) -----------------------------------
        xr = [
            xr_ps.tile([128, 512], F32, tag=f"xr{k}", name=f"xr{k}")
            for k in range(4)
        ]

        p_tiles = {}
        xvs_tiles = {}
        rs4_tiles = {}

        def do_exp(t, c):
            p = p_tiles[t]
            e_t = etiles.pop((t, c))
            if (t, c) in ((0, 0), (0, 1)):
                # split so the very first exp depends on a single 512-col
                # fill (subtile deps): the stream starts ~2 fills earlier.
                nc.scalar.activation(out=p[:, ds(1024 * c, 512)],
                                     in_=e_t[:, 0:512], func=EXP)
                nc.scalar.activation(out=p[:, ds(1024 * c + 512, 512)],
                                     in_=e_t[:, 512:1024], func=EXP)
            else:
                nc.scalar.activation(out=p[:, ds(1024 * c, 1024)], in_=e_t,
                                     func=EXP)
            # rowsum, spread across the tile period and off the scalar
            # queue: chunks 0-2 are folded 1024->512 on the otherwise-idle
            # gpsimd (~1.1us) and reduced on vector (~660ns); chunk 3 is a
            # direct vector reduce (~1.2us) so the tile's rowsum completes
            # one op after its last exp.
            if t not in rs4_tiles:
                rs4_tiles[t] = small.tile([128, 4], F32, tag="rs4", name=f"rs4_{t}")
            rs4 = rs4_tiles[t]
            if c < 3:
                hf = small.tile([128, 512], BF16, tag="hf")
                nc.gpsimd.tensor_add(out=hf, in0=p[:, ds(1024 * c, 512)],
                                     in1=p[:, ds(1024 * c + 512, 512)])
                nc.vector.tensor_reduce(out=rs4[:, c:c + 1], in_=hf,
                                        axis=mybir.AxisListType.X, op=ADD)
            else:
                nc.vector.tensor_reduce(out=rs4[:, 3:4],
                                        in_=p[:, ds(3072, 1024)],
                                        axis=mybir.AxisListType.X, op=ADD)

        def rowsum_tile(t):
            rs4 = rs4_tiles.pop(t)
            rs = small.tile([128, 1], F32, tag="rs")
            nc.vector.tensor_reduce(out=rs, in_=rs4,
                                    axis=mybir.AxisListType.X, op=ADD)
            rr = small.tile([128, 1], F32, tag="rr")
            nc.vector.reciprocal(out=rr, in_=rs)
            xvs = small.tile([128, C4], BF16, tag="xvs")
            nc.vector.tensor_scalar_mul(out=xvs, in0=xvt_sb[t], scalar1=rr)
            xvs_tiles[t] = xvs

        def emit_av(t):
            p = p_tiles[t]
            xvs = xvs_tiles.pop(t)
            first = t == 0
            last = t == NTILES - 1
            # t==0 uses 512-wide AV (start=True must cover the full 2KB
            # PSUM zero-region); mid tiles 256 so an in-flight AV matmul
            # delays a just-released energy fill by at most ~215ns; the
            # last tile uses 512 again (nothing left to delay) emitted in
            # bank order so banks close staggered for the epilogue.
            av_w = 512 if (first or last) else 256
            for j in range(8):
                k, po = j // 2, (j % 2) * 64
                for s in range(512 // av_w):
                    mm = nc.tensor.matmul(
                        xr[k][po:po + 64, ds(s * av_w, av_w)], xvs,
                        p[:, ds(j * 512 + s * av_w, av_w)],
                        start=first, stop=last, tile_position=(0, po),
                        skip_group_check=True,
                    )
                    if not last:
                        mm.ins.bass_priority = 1_000_000 + t * 100 + j * 4 + s

        # ---- the stream --------------------------------------------------
        # AV(t) is emitted one tile late (at (t+1, 3)) so in the in-order
        # PE queue ALL of tile t+1's fills statically precede AV(t): a late
        # xvs(t) can then never stall the exp stream behind an AV group.
        for i, (t, c) in enumerate(chunk_list):
            if t not in p_tiles:
                p_tiles[t] = work.tile([128, N], BF16, tag="p", name=f"p{t}")
            do_exp(t, c)
            if i + 2 < len(chunk_list):
                nt_, nc_ = chunk_list[i + 2]
                etiles[(nt_, nc_)] = emit_fill(nt_, nc_)
            if c == 3:
                rowsum_tile(t)
                if t >= 1 and (t - 1) in xvs_tiles:
                    emit_av(t - 1)
                if t == 1 and 0 in xvs_tiles:
                    emit_av(0)
                if t == NTILES - 1:
                    emit_av(t)

        # ---- epilogue: per-bank staggered PSUM->SBUF copy + DMA ----------
        out_sb = sing.tile([128, 4, 512], F32)
        dma_engines = [nc.sync, nc.gpsimd, nc.scalar, nc.sync]
        for k in range(4):
            if k % 2 == 0:
                nc.scalar.copy(out=out_sb[:, k, :], in_=xr[k])
            else:
                nc.vector.tensor_copy(out=out_sb[:, k, :], in_=xr[k])
            eng = dma_engines[k]
            eng.dma_start(out=out_p[:, ts(2 * k, 512)], in_=out_sb[0:64, k, :])
            eng.dma_start(out=out_p[:, ts(2 * k + 1, 512)],
                          in_=out_sb[64:128, k, :])

    nc.compile()
    return nc


_NC_CACHE = None


def _get_nc():
    global _NC_CACHE
    if _NC_CACHE is None:
        _NC_CACHE = build_nc()
    return _NC_CACHE


def make_in_maps(x, W_qk, W_v, b_v):
    bf = ml_dtypes.bfloat16
    x = np.asarray(x, dtype=np.float32)
    W_qk = np.asarray(W_qk, dtype=np.float32)
    W_v = np.asarray(W_v, dtype=np.float32)
    b_v = np.asarray(b_v, dtype=np.float32)
    xbf = np.ascontiguousarray(x).astype(bf)
    wqt = np.ascontiguousarray((W_qk / np.sqrt(FACTOR)).T).astype(bf)
    wvt = np.ascontiguousarray(W_v.T).astype(bf)
    bvb = np.ascontiguousarray(b_v).astype(bf)
    in_maps = []
    for core in range(8):
        b, h = core // 2, core % 2
        xm = xbf[b] if h == 0 else np.ascontiguousarray(
            np.roll(xbf[b], -NH, axis=1))
        in_maps.append({
            "x_m": xm,
            "wq_t": wqt,
            "wv_t": wvt,
            "bv": bvb,
        })
    return in_maps


def kernel(x, W_qk, W_v, b_v, _trace=False):
    from concourse.bass_utils import run_bass_kernel_spmd

    nc = _get_nc()
    in_maps = make_in_maps(x, W_qk, W_v, b_v)
    res = run_bass_kernel_spmd(nc, in_maps, list(range(8)), trace=_trace)
    if _trace:
        print(f"HW exec time: {res.exec_time_ns} ns")
        print(f"mean exec time: {res.mean_exec_time_ns} ns")
    outs = [res.results[i]["out_p"] for i in range(8)]
    out = np.stack([
        outs[2 * b] + np.roll(outs[2 * b + 1], NH, axis=1) for b in range(B)
    ])
    return out.astype(np.float32)
